# revision 23
# baseline (speedup 1.0000x reference)
"""Trainium2 Bass kernel for nn_Direction_17549236371980 (BEV direction cross-attention).

Strategy: head-parallel over 8 cores (1 head each). Shared preprocessing
(BN+ReLU+1x1 convs, LN stats) replicated per core; attention (scores, softmax,
value mix) sharded by head; per-head outputs AllGathered on-chip; final
proj+MLP+LN stage replicated. All heavy matmuls in bf16 with fp32 PSUM.

Host does only tiny-matrix folding (3x3/4x4 geometry, BN/LN weight folds,
constant grids) and sharding prep.
"""
import numpy as np
import ml_dtypes

_BF16 = ml_dtypes.bfloat16

B, NV, DIM, FD, FH, FW, H, W = 1, 6, 128, 256, 24, 60, 25, 25
HEADS, DH = 8, 64
HD = HEADS * DH
IMG_H, IMG_W = 224, 480
BEV_H, BEV_W, H_M, W_M, OFF = 200, 200, 100.0, 100.0, 0.0
hw = FH * FW          # 1440
Q = H * W             # 625
T = NV * hw           # 8640
EPS = 1e-5

# tiling
NCONV = 3             # 1440 = 3 x 480 per view
CW = hw // NCONV      # 480
SCH = [(i * 512, min(512, T - i * 512)) for i in range((T + 511) // 512)]  # 17 stat chunks
NT = (T + 127) // 128  # 68 t-tiles (last = 64 rows)
TS = [(j * 128, min(128, T - j * 128)) for j in range(NT)]
NPAIR = (NT + 1) // 2  # 34
QS1, QS2 = 512, Q - 512  # 512 + 113 N-split for 625-wide matmuls
QB = 80               # token block per core for the final stage (8*80=640 >= Q)
QPAD = 8 * QB

_CACHE = {}


def _host_prep(x, feature, I_inv, E_inv, p):
    f64 = np.float64
    x = np.asarray(x, f64)[0]
    feature = np.asarray(feature, f64)[0]
    I_inv = np.asarray(I_inv, f64)[0]
    E_inv = np.asarray(E_inv, f64)[0]
    p = {k: np.asarray(v, f64) for k, v in p.items()}

    Xg, Yg = np.meshgrid(np.linspace(0, 1, FW), np.linspace(0, 1, FH))
    pixel3 = np.stack([Xg * IMG_W, Yg * IMG_H, np.ones_like(Xg)], 0).reshape(3, hw)
    Xb, Yb = np.meshgrid(np.linspace(0, 1, W), np.linspace(0, 1, H))
    gb = np.stack([Xb * BEV_W, Yb * BEV_H, np.ones_like(Xb)], 0).reshape(3, Q)
    sh, sw = BEV_H / H_M, BEV_W / W_M
    V = np.array([[0.0, -sw, BEV_W / 2.0], [-sh, 0.0, BEV_H * OFF + BEV_H / 2.0],
                  [0.0, 0.0, 1.0]], f64)
    world = (np.linalg.inv(V) @ gb)[:2]

    Pc = np.eye(DIM) - np.ones((DIM, DIM)) / DIM
    c_embed = np.stack([p['cam_w'] @ E_inv[n][:, 3] for n in range(NV)])

    img_c = np.zeros((DIM, T))
    for n in range(NV):
        A = p['img_w'] @ E_inv[n]
        M3 = A[:, :3] @ I_inv[n]
        M3 = M3.copy()
        M3[:, 2] += A[:, 3] - c_embed[n]
        img_pre = M3 @ pixel3
        nrm = np.sqrt((img_pre ** 2).sum(0))
        img_c[:, n * hw:(n + 1) * hw] = Pc @ (img_pre / (nrm + 1e-7))

    out = {}
    for pre in ['proj', 'lin']:
        s = p[pre + '_bn_g'] / np.sqrt(p[pre + '_bn_v'] + EPS)
        t = p[pre + '_bn_b'] - p[pre + '_bn_m'] * s
        c = t / s
        W_eff = p[pre + '_conv_w'] * s[None, :]
        Wc = Pc @ W_eff
        bias_c = Pc @ (W_eff @ c)
        out[pre + '_negc'] = np.asarray(-c, np.float32).reshape(2, 128).T.copy()  # (128,2)
        # lhsT chunks: (2, 128fd, 128dim)
        out[pre + '_WcT'] = np.ascontiguousarray(
            Wc.T.reshape(2, 128, 128)).astype(_BF16)
        out[pre + '_bias'] = np.asarray(bias_c, np.float32).reshape(128, 1)
    img_c += out['proj_bias'].astype(f64)  # fold key-branch bias into img
    del out['proj_bias']
    out['img_c'] = img_c.astype(_BF16)

    w_embed = p['bev_w'] @ world + p['bev_b'][:, None]
    x2d = x.reshape(DIM, Q)
    qsum_ln = np.zeros((DIM, Q))
    for n in range(2):
        bev_pre = w_embed - c_embed[n][:, None]
        nrm = np.sqrt((bev_pre ** 2).sum(0))
        query = bev_pre / (nrm + 1e-7) + x2d
        mu = query.mean(0)
        var = ((query - mu) ** 2).mean(0)
        qsum_ln += (query - mu) / np.sqrt(var + EPS)
    out['qsum_ln'] = qsum_ln.astype(_BF16)
    x_pad = np.zeros((DIM, QPAD), np.float32)
    x_pad[:, :Q] = x2d
    out['x_pad'] = x_pad

    sc = DH ** -0.5
    heads = []
    for h in range(HEADS):
        sl = slice(h * DH, (h + 1) * DH)
        Wq = p['q_w'][sl] * p['q_ln_g'][None, :]
        qb = 2.0 * (p['q_w'][sl] @ p['q_ln_b'] + p['q_b'][sl])
        Wk = p['k_w'][sl] * p['k_ln_g'][None, :]
        Wv = p['v_w'][sl] * p['v_ln_g'][None, :]
        vb = p['v_b'][sl] + p['v_w'][sl] @ p['v_ln_b']
        heads.append(dict(
            WqT=np.ascontiguousarray((sc * Wq).T).astype(_BF16),       # (128,64)
            qb=np.asarray(sc * qb, np.float32).reshape(64, 1),
            WkT=np.ascontiguousarray(Wk.T).astype(_BF16),              # (128,64)
            WvT=np.ascontiguousarray(Wv.T).astype(_BF16),              # (128,64)
            vb=np.asarray(vb, np.float32).reshape(64, 1),
        ))
    out['heads'] = heads

    # feature (6, 256, hw) -> (6, 2, 128, hw) bf16
    out['feature'] = np.ascontiguousarray(
        feature.reshape(NV, 2, 128, hw)).astype(_BF16)

    # final stage
    out['projT'] = np.ascontiguousarray(p['proj_w'].T.reshape(4, 128, 128)).astype(_BF16)
    out['proj_b'] = np.asarray(p['proj_b'], np.float32).reshape(128, 1)
    out['mlp1T'] = np.ascontiguousarray(
        p['mlp1_w'].T.reshape(128, 2, 128).transpose(1, 0, 2)).astype(_BF16)  # (2,128k,128m)
    out['mlp1_b'] = np.asarray(p['mlp1_b'], np.float32).reshape(2, 128, 1)
    out['mlp2T'] = np.ascontiguousarray(p['mlp2_w'].T.reshape(2, 128, 128)).astype(_BF16)
    out['mlp2_b'] = np.asarray(p['mlp2_b'], np.float32).reshape(128, 1)
    for nm in ['pre_g', 'pre_b', 'post_g', 'post_b']:
        out[nm] = np.asarray(p[nm], np.float32).reshape(128, 1)
    return out


def _build_program():
    import sys
    if '/opt/trn_rl_repo' not in sys.path:
        sys.path.insert(0, '/opt/trn_rl_repo')
    from concourse import bass, bacc, tile, mybir

    f32 = mybir.dt.float32
    bf16 = mybir.dt.bfloat16
    nc = bacc.Bacc(None, target_bir_lowering=False)

    # ---- dram I/O -------------------------------------------------------
    d = {}
    def din(name, shape, dt=f32):
        d[name] = nc.dram_tensor(name, shape, dt, kind="ExternalInput")
        return d[name]

    din('feature', (NV, 2, 128, hw), bf16)
    din('img_c', (DIM, T), bf16)
    din('proj_negc', (128, 2)); din('lin_negc', (128, 2))
    din('proj_WcT', (2, 128, 128), bf16); din('lin_WcT', (2, 128, 128), bf16)
    din('lin_bias', (128, 1))
    din('qsum_ln', (DIM, Q), bf16); din('x_blk', (DIM, QB))
    din('WqT', (128, 64), bf16); din('qb', (64, 1))
    din('WkT', (128, 64), bf16); din('WvT', (128, 64), bf16); din('vb', (64, 1))
    din('projT', (4, 128, 128), bf16); din('proj_b', (128, 1))
    din('mlp1T', (2, 128, 128), bf16); din('mlp1_b', (2, 128, 1))
    din('mlp2T', (2, 128, 128), bf16); din('mlp2_b', (128, 1))
    din('pre_g', (128, 1)); din('pre_b', (128, 1))
    din('post_g', (128, 1)); din('post_b', (128, 1))

    out_d = nc.dram_tensor('out', (DIM, QB), f32, kind="ExternalOutput")
    rowk_d = nc.dram_tensor('rowk', (17 * 512,), f32, kind="Internal")
    rowv_d = nc.dram_tensor('rowv', (17 * 512,), f32, kind="Internal")
    cc_in = nc.dram_tensor('cc_in', (512, QB), bf16, kind="Internal")
    cc_out = nc.dram_tensor('cc_out', (512, QB), bf16, kind="Internal")
    RG = [[0, 1, 2, 3, 4, 5, 6, 7]]

    with tile.TileContext(nc) as tc:
        with (
            tc.tile_pool(name="consts", bufs=1) as consts,
            tc.tile_pool(name="feat", bufs=3) as featp,
            tc.tile_pool(name="fmax", bufs=1) as fmaxp,
            tc.tile_pool(name="big", bufs=1) as bigp,
            tc.tile_pool(name="sq", bufs=3) as sqp,
            tc.tile_pool(name="rows", bufs=1) as rowsp,
            tc.tile_pool(name="epool", bufs=3) as epool,
            tc.tile_pool(name="small", bufs=1) as smallp,
            tc.tile_pool(name="ps_s", bufs=3, space="PSUM") as ps_s,
            tc.tile_pool(name="ps_o", bufs=1, space="PSUM") as ps_o,
        ):
            # ---- constants to SBUF ---------------------------------------
            def load1(name, shape, dt=f32, src=None):
                t = consts.tile(list(shape), dt, tag=name)
                nc.scalar.dma_start(t[:], (src if src is not None else d[name].ap()))
                return t
            def loadch(name, n, dt=bf16):
                # (n, 128, F) dram -> [128, n, F] sbuf; lhsT chunk = t[:, i, :]
                t = consts.tile([128, n, 128], dt, tag=name)
                for i in range(n):
                    nc.scalar.dma_start(t[:, i, :], d[name].ap()[i])
                return t
            proj_negc = consts.tile([128, 2], f32, tag='proj_negc')
            nc.sync.dma_start(proj_negc[:], d['proj_negc'].ap())
            lin_negc = consts.tile([128, 2], f32, tag='lin_negc')
            nc.sync.dma_start(lin_negc[:], d['lin_negc'].ap())
            proj_WcT = consts.tile([128, 2, 128], bf16, tag='proj_WcT')
            lin_WcT = consts.tile([128, 2, 128], bf16, tag='lin_WcT')
            for i in range(2):
                nc.sync.dma_start(proj_WcT[:, i, :], d['proj_WcT'].ap()[i])
                nc.sync.dma_start(lin_WcT[:, i, :], d['lin_WcT'].ap()[i])
            lin_bias = load1('lin_bias', (128, 1))
            qsum_ln = load1('qsum_ln', (DIM, Q), bf16)
            x_blk = load1('x_blk', (DIM, QB))
            WqT = load1('WqT', (128, 64), bf16); qb = load1('qb', (64, 1))
            WkT = load1('WkT', (128, 64), bf16); WvT = load1('WvT', (128, 64), bf16)
            vb = load1('vb', (64, 1))
            projT = loadch('projT', 4)
            proj_b = load1('proj_b', (128, 1))
            mlp1T = loadch('mlp1T', 2)
            mlp1_b = consts.tile([128, 2], f32, tag='mlp1_b')
            for i in range(2):
                nc.sync.dma_start(mlp1_b[:, i:i + 1], d['mlp1_b'].ap()[i])
            mlp2T = loadch('mlp2T', 2)
            mlp2_b = load1('mlp2_b', (128, 1))
            pre_g = load1('pre_g', (128, 1)); pre_b = load1('pre_b', (128, 1))
            post_g = load1('post_g', (128, 1)); post_b = load1('post_b', (128, 1))

            # onehot selector for stat placement: sel[:, c, c'] = 1 iff c==c'
            sel = consts.tile([128, 17, 17], bf16)
            nc.vector.memset(sel[:], 0.0)
            for c in range(17):
                nc.vector.memset(sel[:, c, c:c + 1], 1.0)
            ones1x64 = consts.tile([1, 64], f32)
            nc.vector.memset(ones1x64[:], 1.0)
            ones1x128 = consts.tile([1, 128], f32)
            nc.vector.memset(ones1x128[:], 1.0)
            ones128x1_f32 = consts.tile([128, 1], f32)
            nc.vector.memset(ones128x1_f32[:], 1.0)
            eps17 = consts.tile([17, 1], f32)
            nc.vector.memset(eps17[:], EPS)
            eps1 = consts.tile([1, 1], f32)
            nc.vector.memset(eps1[:], EPS)

            # ---- phase 1: BN+ReLU + convs -> key_c / val_c ---------------
            key_c = bigp.tile([DIM, T], bf16, tag="key_c")
            val_c = bigp.tile([DIM, T], bf16, tag="val_c")
            for n in range(NV):
                fmx = {'proj': [], 'lin': []}
                for c in range(2):
                    ftile = featp.tile([128, hw], bf16, tag="ftile")
                    nc.sync.dma_start(ftile[:], d['feature'].ap()[n, c])
                    for pre, negc in [('proj', proj_negc), ('lin', lin_negc)]:
                        fm = fmaxp.tile([128, hw], bf16, tag=f"fmax_{pre}{c}")
                        nc.vector.tensor_scalar_max(fm[:], ftile[:], negc[:, c:c + 1])
                        fmx[pre].append(fm)
                for (k0, kn) in [(0, 512), (512, 512), (1024, 416)]:
                    ksl = slice(k0, k0 + kn)
                    osl = slice(n * hw + k0, n * hw + k0 + kn)
                    pk = ps_s.tile([128, 512], f32, tag="s")
                    pv = ps_s.tile([128, 512], f32, tag="s")
                    for c in range(2):
                        nc.tensor.matmul(pk[:, :kn], proj_WcT[:, c, :],
                                         fmx['proj'][c][:, ksl],
                                         start=(c == 0), stop=(c == 1))
                    for c in range(2):
                        nc.tensor.matmul(pv[:, :kn], lin_WcT[:, c, :],
                                         fmx['lin'][c][:, ksl],
                                         start=(c == 0), stop=(c == 1))
                    imgt = featp.tile([128, 512], bf16, tag="imgt")
                    nc.sync.dma_start(imgt[:, :kn], d['img_c'].ap()[:, osl])
                    nc.vector.tensor_add(key_c[:, osl], pk[:, :kn], imgt[:, :kn])
                    nc.scalar.activation(val_c[:, osl], pv[:, :kn],
                                         mybir.ActivationFunctionType.Identity,
                                         bias=lin_bias[:])

            # ---- phase 2: column stats (sumsq over DIM) -> rinv rows -----
            def col_rinv(src, rowbuf_d):
                pstat = ps_s.tile([17, 512], f32, tag="s")
                for c, (c0, cn) in enumerate(SCH):
                    sq = sqp.tile([128, 512], bf16, tag="sq")
                    nc.scalar.activation(sq[:, :cn], src[:, c0:c0 + cn],
                                         mybir.ActivationFunctionType.Square)
                    nc.tensor.matmul(pstat[:, :cn], sel[:, c], sq[:, :cn],
                                     start=(c == 0), stop=(c == len(SCH) - 1))
                sig = rowsp.tile([17, 512], f32, tag="sig")
                # sig = sqrt(sumsq/128 + eps)
                nc.scalar.activation(sig[:], pstat[:],
                                     mybir.ActivationFunctionType.Sqrt,
                                     bias=eps17[:], scale=1.0 / DIM)
                rinv = rowsp.tile([17, 512], f32, tag="rinv")
                nc.vector.reciprocal(rinv[:], sig[:])
                nc.sync.dma_start(rowbuf_d.ap().rearrange("(a b) -> a b", a=17),
                                  rinv[:])
            col_rinv(key_c, rowk_d)
            col_rinv(val_c, rowv_d)

            # broadcast rinv rows across partitions (with f32->bf16 cast)
            def bcast(rowbuf_d, nparts):
                t = bigp.tile([nparts, T], bf16, tag=f"bc_{rowbuf_d.name}")
                src = bass.AP(tensor=rowbuf_d.ap().tensor, offset=0,
                              ap=[[0, nparts], [1, T]])
                nc.gpsimd.dma_start(out=t[:], in_=src)
                return t
            rinvk_b = bcast(rowk_d, 64)
            rinvv_b = bcast(rowv_d, 128)

            # ---- phase 3: val_ln, k-proj (kp), qs ------------------------
            val_ln = val_c
            for c, (c0, cn) in enumerate(SCH):
                nc.vector.tensor_mul(val_ln[:, c0:c0 + cn], val_c[:, c0:c0 + cn],
                                     rinvv_b[:, c0:c0 + cn])

            kp = bigp.tile([64, T], bf16, tag="kp")
            for c, (c0, cn) in enumerate(SCH):
                pp = ps_s.tile([64, 512], f32, tag="s")
                nc.tensor.matmul(pp[:, :cn], WkT[:], key_c[:, c0:c0 + cn],
                                 start=True, stop=True)
                nc.vector.tensor_mul(kp[:, c0:c0 + cn], pp[:, :cn],
                                     rinvk_b[:, c0:c0 + cn])

            qs = smallp.tile([64, Q], bf16, tag="qs")
            pq = ps_o.tile([64, Q], f32, tag="o")
            nc.tensor.matmul(pq[:, :QS1], WqT[:], qsum_ln[:, :QS1], start=True, stop=True)
            nc.tensor.matmul(pq[:, QS1:], WqT[:], qsum_ln[:, QS1:], start=True, stop=True)
            nc.vector.tensor_scalar_add(qs[:], pq[:], qb[:])

            # ---- phase 4: vp tiles (token-major values + ones col) -------
            vp = bigp.tile([128, NT, 65], bf16, tag="vp")
            for j, (t0, tn) in enumerate(TS):
                pvp = ps_s.tile([128, 64], f32, tag="s")
                nc.tensor.matmul(pvp[:tn, :], val_ln[:, t0:t0 + tn], WvT[:],
                                 start=True, stop=True)
                nc.vector.tensor_copy(vp[:tn, j, 0:64], pvp[:tn, :])
                nc.vector.memset(vp[:tn, j, 64:65], 1.0)

            # ---- phase 5: attention: S = kp^T qs ; E = exp(S); O += vp^T E
            po = ps_o.tile([65, Q], f32, tag="o")
            for j in range(NT):
                t0, tn = TS[j]
                ps = ps_s.tile([128, Q], f32, tag="s")
                ep = epool.tile([128, Q], bf16, tag="ep")
                nc.tensor.matmul(ps[:tn, :QS1], kp[:, t0:t0 + tn],
                                 qs[:, :QS1], start=True, stop=True)
                nc.tensor.matmul(ps[:tn, QS1:], kp[:, t0:t0 + tn],
                                 qs[:, QS1:], start=True, stop=True)
                nc.scalar.activation(ep[:tn, :], ps[:tn, :],
                                     mybir.ActivationFunctionType.Exp)
                first = (j == 0)
                last = (j == NT - 1)
                nc.tensor.matmul(po[:, :QS1], vp[:tn, j, :], ep[:tn, :QS1],
                                 start=first, stop=last, skip_group_check=True)
                nc.tensor.matmul(po[:, QS1:], vp[:tn, j, :], ep[:tn, QS1:],
                                 start=first, stop=last, skip_group_check=True)

            # ---- phase 6: divide by sumE, add vb, AllGather --------------
            recip = smallp.tile([1, Q], f32, tag="recip")
            nc.vector.reciprocal(recip[:], po[64:65, :])
            prb = ps_s.tile([64, Q], f32, tag="s")
            nc.tensor.matmul(prb[:, :QS1], ones1x64[:], recip[:, :QS1],
                             start=True, stop=True)
            nc.tensor.matmul(prb[:, QS1:], ones1x64[:], recip[:, QS1:],
                             start=True, stop=True)
            recb = smallp.tile([64, Q], f32, tag="recb")
            nc.vector.tensor_copy(recb[:], prb[:])
            ahead = smallp.tile([64, QPAD], bf16, tag="ahead")
            nc.vector.tensor_mul(ahead[:, :Q], po[0:64, :], recb[:])
            nc.vector.tensor_scalar_add(ahead[:, :Q], ahead[:, :Q], vb[:])
            nc.vector.memset(ahead[:, Q:], 0.0)
            cc_in_v = cc_in.ap().rearrange("(a p) f -> a p f", p=64)
            for j in range(8):
                nc.sync.dma_start(cc_in_v[j], ahead[:, j * QB:(j + 1) * QB])
            nc.gpsimd.collective_compute(
                "AllToAll", mybir.AluOpType.bypass,
                ins=[cc_in[:]], outs=[cc_out[:]], replica_groups=RG,
            )

            # ---- phase 7: final stage (replicated) -----------------------
            cc_view = cc_out.ap().rearrange("(a p) f -> a p f", p=128)
            ats = []
            for i in range(4):
                at = smallp.tile([128, QB], bf16, tag=f"at{i}")
                nc.sync.dma_start(at[:], cc_view[i])
                ats.append(at)

            pz = ps_o.tile([128, QB], f32, tag="o")
            for i in range(4):
                nc.tensor.matmul(pz[:], projT[:, i, :], ats[i][:],
                                 start=(i == 0), stop=(i == 3), skip_group_check=True)
            z1 = smallp.tile([128, QB], f32, tag="z1")
            nc.vector.tensor_add(z1[:], pz[:], x_blk[:])
            nc.vector.tensor_scalar_add(z1[:], z1[:], proj_b[:])

            def part_ln(z, g, b, out_dt, out_tag):
                """LayerNorm over the partition axis of z (128, QB)."""
                zsq = smallp.tile([128, QB], f32, tag="ln_sq")
                nc.vector.tensor_mul(zsq[:], z[:], z[:])
                p_sq = ps_s.tile([1, 2 * QB], f32, tag="s")
                p_s = p_sq[:, 0:QB]
                p_q = p_sq[:, QB:2 * QB]
                nc.tensor.matmul(p_s[:], ones128x1_f32[:], z[:],
                                 start=True, stop=True, skip_group_check=True)
                nc.tensor.matmul(p_q[:], ones128x1_f32[:], zsq[:],
                                 start=True, stop=True, skip_group_check=True)
                mu = smallp.tile([1, QB], f32, tag="ln_mu")
                nc.vector.tensor_scalar_mul(mu[:], p_s[:], 1.0 / DIM)
                musq = smallp.tile([1, QB], f32, tag="ln_musq")
                nc.vector.tensor_mul(musq[:], mu[:], mu[:])
                var = smallp.tile([1, QB], f32, tag="ln_var")
                nc.vector.tensor_scalar_mul(var[:], p_q[:], 1.0 / DIM)
                nc.vector.tensor_sub(var[:], var[:], musq[:])
                sig = smallp.tile([1, QB], f32, tag="ln_sig")
                nc.scalar.activation(sig[:], var[:],
                                     mybir.ActivationFunctionType.Sqrt, bias=eps1[:])
                rinv = smallp.tile([1, QB], f32, tag="ln_rinv")
                nc.vector.reciprocal(rinv[:], sig[:])
                mur = smallp.tile([1, QB], f32, tag="ln_mur")
                nc.vector.tensor_mul(mur[:], mu[:], rinv[:])
                pbb = ps_s.tile([128, 2 * QB], f32, tag="s")
                pb1 = pbb[:, 0:QB]
                pb2 = pbb[:, QB:2 * QB]
                nc.tensor.matmul(pb1[:], ones1x128[:], rinv[:], start=True, stop=True)
                nc.tensor.matmul(pb2[:], ones1x128[:], mur[:], start=True, stop=True)
                t1 = smallp.tile([128, QB], f32, tag="ln_t1")
                nc.vector.tensor_mul(t1[:], pb1[:], z[:])
                t2 = smallp.tile([128, QB], f32, tag="ln_t2")
                nc.vector.tensor_copy(t2[:], pb2[:])
                zl = smallp.tile([128, QB], out_dt, tag=out_tag)
                nc.vector.tensor_sub(t1[:], t1[:], t2[:])
                nc.vector.tensor_scalar(zl[:], t1[:], g[:], b[:],
                                        mybir.AluOpType.mult, mybir.AluOpType.add)
                return zl

            z_ln = part_ln(z1, pre_g, pre_b, f32, "zln")
            z_ln_bf = smallp.tile([128, QB], bf16, tag="zlnbf")
            nc.vector.tensor_copy(z_ln_bf[:], z_ln[:])

            h1s = []
            for j in range(2):
                ph = ps_s.tile([128, QB], f32, tag="s")
                nc.tensor.matmul(ph[:], mlp1T[:, j, :], z_ln_bf[:],
                                 start=True, stop=True)
                h1 = smallp.tile([128, QB], bf16, tag=f"h1{j}")
                nc.scalar.activation(h1[:], ph[:],
                                     mybir.ActivationFunctionType.Gelu,
                                     bias=mlp1_b[:, j:j + 1])
                h1s.append(h1)
            pz2 = ps_o.tile([128, QB], f32, tag="o")
            for j in range(2):
                nc.tensor.matmul(pz2[:], mlp2T[:, j, :], h1s[j][:],
                                 start=(j == 0), stop=(j == 1), skip_group_check=True)
            z2 = smallp.tile([128, QB], f32, tag="z2")
            nc.vector.tensor_scalar_add(z2[:], pz2[:], mlp2_b[:])
            nc.vector.tensor_add(z2[:], z2[:], z_ln[:])

            z_out = part_ln(z2, post_g, post_b, f32, "zout")
            nc.sync.dma_start(out_d.ap(), z_out[:])

    nc.compile()
    return nc


def _make_in_maps(dev):
    shared = {k: dev[k] for k in [
        'feature', 'img_c', 'proj_negc', 'lin_negc', 'proj_WcT', 'lin_WcT',
        'lin_bias', 'qsum_ln', 'projT', 'proj_b', 'mlp1T', 'mlp1_b',
        'mlp2T', 'mlp2_b', 'pre_g', 'pre_b', 'post_g', 'post_b']}
    in_maps = []
    for h in range(HEADS):
        m = dict(shared)
        m.update(dev['heads'][h])
        m['x_blk'] = np.ascontiguousarray(dev['x_pad'][:, h * QB:(h + 1) * QB])
        in_maps.append(m)
    return in_maps


def kernel(x, feature, I_inv, E_inv, params):
    import sys
    if '/opt/trn_rl_repo' not in sys.path:
        sys.path.insert(0, '/opt/trn_rl_repo')
    from concourse.bass_utils import run_bass_kernel_spmd

    dev = _host_prep(x, feature, I_inv, E_inv, params)
    if 'nc' not in _CACHE:
        _CACHE['nc'] = _build_program()
    nc = _CACHE['nc']

    in_maps = _make_in_maps(dev)

    res = run_bass_kernel_spmd(nc, in_maps, core_ids=list(range(8)))
    z = np.concatenate([np.asarray(res.results[c]['out'], np.float32)
                        for c in range(HEADS)], axis=1)[:, :Q]
    return z.reshape(1, DIM, H, W)


# revision 24
# speedup vs baseline: 1.0755x; 1.0755x over previous
"""Trainium2 Bass kernel for nn_Direction_17549236371980 (BEV direction cross-attention).

Strategy: head-parallel over 8 cores (1 head each). Shared preprocessing
(BN+ReLU+1x1 convs, LN stats) replicated per core; attention (scores, softmax,
value mix) sharded by head; per-head outputs AllGathered on-chip; final
proj+MLP+LN stage replicated. All heavy matmuls in bf16 with fp32 PSUM.

Host does only tiny-matrix folding (3x3/4x4 geometry, BN/LN weight folds,
constant grids) and sharding prep.
"""
import numpy as np
import ml_dtypes

_BF16 = ml_dtypes.bfloat16

B, NV, DIM, FD, FH, FW, H, W = 1, 6, 128, 256, 24, 60, 25, 25
HEADS, DH = 8, 64
HD = HEADS * DH
IMG_H, IMG_W = 224, 480
BEV_H, BEV_W, H_M, W_M, OFF = 200, 200, 100.0, 100.0, 0.0
hw = FH * FW          # 1440
Q = H * W             # 625
T = NV * hw           # 8640
EPS = 1e-5

# tiling
NCONV = 3             # 1440 = 3 x 480 per view
CW = hw // NCONV      # 480
SCH = [(i * 512, min(512, T - i * 512)) for i in range((T + 511) // 512)]  # 17 stat chunks
NT = (T + 127) // 128  # 68 t-tiles (last = 64 rows)
TS = [(j * 128, min(128, T - j * 128)) for j in range(NT)]
NPAIR = (NT + 1) // 2  # 34
QS1, QS2 = 512, Q - 512  # 512 + 113 N-split for 625-wide matmuls
QB = 80               # token block per core for the final stage (8*80=640 >= Q)
QPAD = 8 * QB

_CACHE = {}


def _host_prep(x, feature, I_inv, E_inv, p):
    f64 = np.float64
    x = np.asarray(x, f64)[0]
    feature = np.asarray(feature, f64)[0]
    I_inv = np.asarray(I_inv, f64)[0]
    E_inv = np.asarray(E_inv, f64)[0]
    p = {k: np.asarray(v, f64) for k, v in p.items()}

    Xg, Yg = np.meshgrid(np.linspace(0, 1, FW), np.linspace(0, 1, FH))
    pixel3 = np.stack([Xg * IMG_W, Yg * IMG_H, np.ones_like(Xg)], 0).reshape(3, hw)
    Xb, Yb = np.meshgrid(np.linspace(0, 1, W), np.linspace(0, 1, H))
    gb = np.stack([Xb * BEV_W, Yb * BEV_H, np.ones_like(Xb)], 0).reshape(3, Q)
    sh, sw = BEV_H / H_M, BEV_W / W_M
    V = np.array([[0.0, -sw, BEV_W / 2.0], [-sh, 0.0, BEV_H * OFF + BEV_H / 2.0],
                  [0.0, 0.0, 1.0]], f64)
    world = (np.linalg.inv(V) @ gb)[:2]

    Pc = np.eye(DIM) - np.ones((DIM, DIM)) / DIM
    c_embed = np.stack([p['cam_w'] @ E_inv[n][:, 3] for n in range(NV)])

    img_c = np.zeros((DIM, T))
    for n in range(NV):
        A = p['img_w'] @ E_inv[n]
        M3 = A[:, :3] @ I_inv[n]
        M3 = M3.copy()
        M3[:, 2] += A[:, 3] - c_embed[n]
        img_pre = M3 @ pixel3
        nrm = np.sqrt((img_pre ** 2).sum(0))
        img_c[:, n * hw:(n + 1) * hw] = Pc @ (img_pre / (nrm + 1e-7))

    out = {}
    for pre in ['proj', 'lin']:
        s = p[pre + '_bn_g'] / np.sqrt(p[pre + '_bn_v'] + EPS)
        t = p[pre + '_bn_b'] - p[pre + '_bn_m'] * s
        c = t / s
        W_eff = p[pre + '_conv_w'] * s[None, :]
        Wc = Pc @ W_eff
        bias_c = Pc @ (W_eff @ c)
        out[pre + '_negc'] = np.asarray(-c, np.float32).reshape(2, 128).T.copy()  # (128,2)
        # lhsT chunks: (2, 128fd, 128dim)
        out[pre + '_WcT'] = np.ascontiguousarray(
            Wc.T.reshape(2, 128, 128)).astype(_BF16)
        out[pre + '_bias'] = np.asarray(bias_c, np.float32).reshape(128, 1)
    img_c += out['proj_bias'].astype(f64)  # fold key-branch bias into img
    del out['proj_bias']
    out['img_c'] = img_c.astype(_BF16)

    w_embed = p['bev_w'] @ world + p['bev_b'][:, None]
    x2d = x.reshape(DIM, Q)
    qsum_ln = np.zeros((DIM, Q))
    for n in range(2):
        bev_pre = w_embed - c_embed[n][:, None]
        nrm = np.sqrt((bev_pre ** 2).sum(0))
        query = bev_pre / (nrm + 1e-7) + x2d
        mu = query.mean(0)
        var = ((query - mu) ** 2).mean(0)
        qsum_ln += (query - mu) / np.sqrt(var + EPS)
    out['qsum_ln'] = qsum_ln.astype(_BF16)
    x_pad = np.zeros((DIM, QPAD), np.float32)
    x_pad[:, :Q] = x2d
    out['x_pad'] = x_pad

    sc = DH ** -0.5
    heads = []
    for h in range(HEADS):
        sl = slice(h * DH, (h + 1) * DH)
        Wq = p['q_w'][sl] * p['q_ln_g'][None, :]
        qb = 2.0 * (p['q_w'][sl] @ p['q_ln_b'] + p['q_b'][sl])
        Wk = p['k_w'][sl] * p['k_ln_g'][None, :]
        Wv = p['v_w'][sl] * p['v_ln_g'][None, :]
        vb = p['v_b'][sl] + p['v_w'][sl] @ p['v_ln_b']
        heads.append(dict(
            WqT=np.ascontiguousarray((sc * Wq).T).astype(_BF16),       # (128,64)
            qb=np.asarray(sc * qb, np.float32).reshape(64, 1),
            WkT=np.ascontiguousarray(Wk.T).astype(_BF16),              # (128,64)
            WvT=np.ascontiguousarray(Wv.T).astype(_BF16),              # (128,64)
            vb=np.asarray(vb, np.float32).reshape(64, 1),
        ))
    out['heads'] = heads

    # feature (6, 256, hw) -> (6, 2, 128, hw) bf16
    out['feature'] = np.ascontiguousarray(
        feature.reshape(NV, 2, 128, hw)).astype(_BF16)

    # final stage
    out['projT'] = np.ascontiguousarray(p['proj_w'].T.reshape(4, 128, 128)).astype(_BF16)
    out['proj_b'] = np.asarray(p['proj_b'], np.float32).reshape(128, 1)
    out['mlp1T'] = np.ascontiguousarray(
        p['mlp1_w'].T.reshape(128, 2, 128).transpose(1, 0, 2)).astype(_BF16)  # (2,128k,128m)
    out['mlp1_b'] = np.asarray(p['mlp1_b'], np.float32).reshape(2, 128, 1)
    out['mlp2T'] = np.ascontiguousarray(p['mlp2_w'].T.reshape(2, 128, 128)).astype(_BF16)
    out['mlp2_b'] = np.asarray(p['mlp2_b'], np.float32).reshape(128, 1)
    for nm in ['pre_g', 'pre_b', 'post_g', 'post_b']:
        out[nm] = np.asarray(p[nm], np.float32).reshape(128, 1)
    return out


def _build_program():
    import sys
    if '/opt/trn_rl_repo' not in sys.path:
        sys.path.insert(0, '/opt/trn_rl_repo')
    from concourse import bass, bacc, tile, mybir

    f32 = mybir.dt.float32
    bf16 = mybir.dt.bfloat16
    nc = bacc.Bacc(None, target_bir_lowering=False)

    # ---- dram I/O -------------------------------------------------------
    d = {}
    def din(name, shape, dt=f32):
        d[name] = nc.dram_tensor(name, shape, dt, kind="ExternalInput")
        return d[name]

    din('feature', (NV, 2, 128, hw), bf16)
    din('img_c', (DIM, T), bf16)
    din('proj_negc', (128, 2)); din('lin_negc', (128, 2))
    din('proj_WcT', (2, 128, 128), bf16); din('lin_WcT', (2, 128, 128), bf16)
    din('lin_bias', (128, 1))
    din('qsum_ln', (DIM, Q), bf16); din('x_blk', (DIM, QB))
    din('WqT', (128, 64), bf16); din('qb', (64, 1))
    din('WkT', (128, 64), bf16); din('WvT', (128, 64), bf16); din('vb', (64, 1))
    din('projT', (4, 128, 128), bf16); din('proj_b', (128, 1))
    din('mlp1T', (2, 128, 128), bf16); din('mlp1_b', (2, 128, 1))
    din('mlp2T', (2, 128, 128), bf16); din('mlp2_b', (128, 1))
    din('pre_g', (128, 1)); din('pre_b', (128, 1))
    din('post_g', (128, 1)); din('post_b', (128, 1))

    out_d = nc.dram_tensor('out', (DIM, QB), f32, kind="ExternalOutput")
    rowk_d = nc.dram_tensor('rowk', (17 * 512,), f32, kind="Internal")
    rowv_d = nc.dram_tensor('rowv', (17 * 512,), f32, kind="Internal")
    cc_in = nc.dram_tensor('cc_in', (512, QB), bf16, kind="Internal")
    cc_out = nc.dram_tensor('cc_out', (512, QB), bf16, kind="Internal")
    RG = [[0, 1, 2, 3, 4, 5, 6, 7]]

    with tile.TileContext(nc) as tc:
        with (
            tc.tile_pool(name="consts", bufs=1) as consts,
            tc.tile_pool(name="feat", bufs=3) as featp,
            tc.tile_pool(name="fmax", bufs=1) as fmaxp,
            tc.tile_pool(name="big", bufs=1) as bigp,
            tc.tile_pool(name="sq", bufs=3) as sqp,
            tc.tile_pool(name="rows", bufs=1) as rowsp,
            tc.tile_pool(name="epool", bufs=3) as epool,
            tc.tile_pool(name="small", bufs=1) as smallp,
            tc.tile_pool(name="ps_s", bufs=2, space="PSUM") as ps_s,
            tc.tile_pool(name="ps_o", bufs=1, space="PSUM") as ps_o,
        ):
            # ---- constants to SBUF ---------------------------------------
            def load1(name, shape, dt=f32, src=None):
                t = consts.tile(list(shape), dt, tag=name)
                nc.scalar.dma_start(t[:], (src if src is not None else d[name].ap()))
                return t
            def loadch(name, n, dt=bf16):
                # (n, 128, F) dram -> [128, n, F] sbuf; lhsT chunk = t[:, i, :]
                t = consts.tile([128, n, 128], dt, tag=name)
                for i in range(n):
                    nc.scalar.dma_start(t[:, i, :], d[name].ap()[i])
                return t
            proj_negc = consts.tile([128, 2], f32, tag='proj_negc')
            nc.sync.dma_start(proj_negc[:], d['proj_negc'].ap())
            lin_negc = consts.tile([128, 2], f32, tag='lin_negc')
            nc.sync.dma_start(lin_negc[:], d['lin_negc'].ap())
            proj_WcT = consts.tile([128, 2, 128], bf16, tag='proj_WcT')
            lin_WcT = consts.tile([128, 2, 128], bf16, tag='lin_WcT')
            for i in range(2):
                nc.sync.dma_start(proj_WcT[:, i, :], d['proj_WcT'].ap()[i])
                nc.sync.dma_start(lin_WcT[:, i, :], d['lin_WcT'].ap()[i])
            lin_bias = load1('lin_bias', (128, 1))
            qsum_ln = load1('qsum_ln', (DIM, Q), bf16)
            x_blk = load1('x_blk', (DIM, QB))
            WqT = load1('WqT', (128, 64), bf16); qb = load1('qb', (64, 1))
            WkT = load1('WkT', (128, 64), bf16); WvT = load1('WvT', (128, 64), bf16)
            vb = load1('vb', (64, 1))
            projT = loadch('projT', 4)
            proj_b = load1('proj_b', (128, 1))
            mlp1T = loadch('mlp1T', 2)
            mlp1_b = consts.tile([128, 2], f32, tag='mlp1_b')
            for i in range(2):
                nc.sync.dma_start(mlp1_b[:, i:i + 1], d['mlp1_b'].ap()[i])
            mlp2T = loadch('mlp2T', 2)
            mlp2_b = load1('mlp2_b', (128, 1))
            pre_g = load1('pre_g', (128, 1)); pre_b = load1('pre_b', (128, 1))
            post_g = load1('post_g', (128, 1)); post_b = load1('post_b', (128, 1))

            # onehot selector for stat placement: sel[:, c, c'] = 1 iff c==c'
            sel = consts.tile([128, 17, 17], bf16)
            nc.vector.memset(sel[:], 0.0)
            for c in range(17):
                nc.vector.memset(sel[:, c, c:c + 1], 1.0)
            ones1x64 = consts.tile([1, 64], f32)
            nc.vector.memset(ones1x64[:], 1.0)
            ones1x128 = consts.tile([1, 128], f32)
            nc.vector.memset(ones1x128[:], 1.0)
            ones128x1_f32 = consts.tile([128, 1], f32)
            nc.vector.memset(ones128x1_f32[:], 1.0)
            eps17 = consts.tile([17, 1], f32)
            nc.vector.memset(eps17[:], EPS)
            eps1 = consts.tile([1, 1], f32)
            nc.vector.memset(eps1[:], EPS)

            # ---- phase 1: BN+ReLU + convs -> key_c / val_c ---------------
            key_c = bigp.tile([DIM, T], bf16, tag="key_c")
            val_c = bigp.tile([DIM, T], bf16, tag="val_c")
            for n in range(NV):
                fmx = {'proj': [], 'lin': []}
                for c in range(2):
                    ftile = featp.tile([128, hw], bf16, tag="ftile")
                    nc.sync.dma_start(ftile[:], d['feature'].ap()[n, c])
                    for pre, negc in [('proj', proj_negc), ('lin', lin_negc)]:
                        fm = fmaxp.tile([128, hw], bf16, tag=f"fmax_{pre}{c}")
                        nc.vector.tensor_scalar_max(fm[:], ftile[:], negc[:, c:c + 1])
                        fmx[pre].append(fm)
                pk = ps_s.tile([128, hw], f32, tag="s")
                pv = ps_s.tile([128, hw], f32, tag="s")
                for (k0, kn) in [(0, 512), (512, 512), (1024, 416)]:
                    ksl = slice(k0, k0 + kn)
                    for c in range(2):
                        nc.tensor.matmul(pk[:, ksl], proj_WcT[:, c, :],
                                         fmx['proj'][c][:, ksl],
                                         start=(c == 0), stop=(c == 1),
                                         skip_group_check=True)
                    for c in range(2):
                        nc.tensor.matmul(pv[:, ksl], lin_WcT[:, c, :],
                                         fmx['lin'][c][:, ksl],
                                         start=(c == 0), stop=(c == 1),
                                         skip_group_check=True)
                osl = slice(n * hw, (n + 1) * hw)
                imgt = featp.tile([128, hw], bf16, tag="imgt")
                nc.sync.dma_start(imgt[:], d['img_c'].ap()[:, osl])
                nc.vector.tensor_add(key_c[:, osl], pk[:], imgt[:])
                nc.scalar.activation(val_c[:, osl], pv[:],
                                     mybir.ActivationFunctionType.Identity,
                                     bias=lin_bias[:])

            # ---- phase 2: column stats (sumsq over DIM) -> rinv rows -----
            def col_rinv(src, rowbuf_d):
                pstat = ps_s.tile([17, 512], f32, tag="s")
                for c, (c0, cn) in enumerate(SCH):
                    sq = sqp.tile([128, 512], bf16, tag="sq")
                    nc.scalar.activation(sq[:, :cn], src[:, c0:c0 + cn],
                                         mybir.ActivationFunctionType.Square)
                    nc.tensor.matmul(pstat[:, :cn], sel[:, c], sq[:, :cn],
                                     start=(c == 0), stop=(c == len(SCH) - 1))
                sig = rowsp.tile([17, 512], f32, tag="sig")
                # sig = sqrt(sumsq/128 + eps)
                nc.scalar.activation(sig[:], pstat[:],
                                     mybir.ActivationFunctionType.Sqrt,
                                     bias=eps17[:], scale=1.0 / DIM)
                rinv = rowsp.tile([17, 512], f32, tag="rinv")
                nc.vector.reciprocal(rinv[:], sig[:])
                nc.sync.dma_start(rowbuf_d.ap().rearrange("(a b) -> a b", a=17),
                                  rinv[:])
            col_rinv(key_c, rowk_d)
            col_rinv(val_c, rowv_d)

            # broadcast rinv rows across partitions (with f32->bf16 cast)
            def bcast(rowbuf_d, nparts):
                t = bigp.tile([nparts, T], bf16, tag=f"bc_{rowbuf_d.name}")
                src = bass.AP(tensor=rowbuf_d.ap().tensor, offset=0,
                              ap=[[0, nparts], [1, T]])
                nc.gpsimd.dma_start(out=t[:], in_=src)
                return t
            rinvk_b = bcast(rowk_d, 64)
            rinvv_b = bcast(rowv_d, 128)

            # ---- phase 3: val_ln, k-proj (kp), qs ------------------------
            val_ln = val_c
            for c, (c0, cn) in enumerate(SCH):
                nc.vector.tensor_mul(val_ln[:, c0:c0 + cn], val_c[:, c0:c0 + cn],
                                     rinvv_b[:, c0:c0 + cn])

            kp = bigp.tile([64, T], bf16, tag="kp")
            for c, (c0, cn) in enumerate(SCH):
                pp = ps_s.tile([64, 512], f32, tag="s")
                nc.tensor.matmul(pp[:, :cn], WkT[:], key_c[:, c0:c0 + cn],
                                 start=True, stop=True)
                nc.vector.tensor_mul(kp[:, c0:c0 + cn], pp[:, :cn],
                                     rinvk_b[:, c0:c0 + cn])

            qs = smallp.tile([64, Q], bf16, tag="qs")
            pq = ps_o.tile([64, Q], f32, tag="o")
            nc.tensor.matmul(pq[:, :QS1], WqT[:], qsum_ln[:, :QS1], start=True, stop=True)
            nc.tensor.matmul(pq[:, QS1:], WqT[:], qsum_ln[:, QS1:], start=True, stop=True)
            nc.vector.tensor_scalar_add(qs[:], pq[:], qb[:])

            # ---- phase 4: vp tiles (token-major values + ones col) -------
            vp = bigp.tile([128, NT, 65], bf16, tag="vp")
            for j, (t0, tn) in enumerate(TS):
                pvp = ps_s.tile([128, 64], f32, tag="s")
                nc.tensor.matmul(pvp[:tn, :], val_ln[:, t0:t0 + tn], WvT[:],
                                 start=True, stop=True)
                nc.vector.tensor_copy(vp[:tn, j, 0:64], pvp[:tn, :])
                nc.vector.memset(vp[:tn, j, 64:65], 1.0)

            # ---- phase 5: attention: S = kp^T qs ; E = exp(S); O += vp^T E
            po = ps_o.tile([65, Q], f32, tag="o")
            for pair in range(NPAIR):
                ja, jb = 2 * pair, 2 * pair + 1
                ps = ps_s.tile([128, 2 * Q], f32, tag="s")
                ep = epool.tile([128, 2 * Q], bf16, tag="ep")
                for half, j in enumerate([ja, jb]):
                    if j >= NT:
                        continue
                    t0, tn = TS[j]
                    base = half * Q
                    nc.tensor.matmul(ps[:tn, base:base + QS1], kp[:, t0:t0 + tn],
                                     qs[:, :QS1], start=True, stop=True)
                    nc.tensor.matmul(ps[:tn, base + QS1:base + Q], kp[:, t0:t0 + tn],
                                     qs[:, QS1:], start=True, stop=True)
                nc.scalar.activation(ep[:], ps[:], mybir.ActivationFunctionType.Exp)
                for half, j in enumerate([ja, jb]):
                    if j >= NT:
                        continue
                    t0, tn = TS[j]
                    base = half * Q
                    first = (j == 0)
                    last = (j == NT - 1)
                    nc.tensor.matmul(po[:, :QS1], vp[:tn, j, :],
                                     ep[:tn, base:base + QS1],
                                     start=first, stop=last, skip_group_check=True)
                    nc.tensor.matmul(po[:, QS1:], vp[:tn, j, :],
                                     ep[:tn, base + QS1:base + Q],
                                     start=first, stop=last, skip_group_check=True)

            # ---- phase 6: divide by sumE, add vb, AllGather --------------
            recip = smallp.tile([1, Q], f32, tag="recip")
            nc.vector.reciprocal(recip[:], po[64:65, :])
            prb = ps_s.tile([64, Q], f32, tag="s")
            nc.tensor.matmul(prb[:, :QS1], ones1x64[:], recip[:, :QS1],
                             start=True, stop=True)
            nc.tensor.matmul(prb[:, QS1:], ones1x64[:], recip[:, QS1:],
                             start=True, stop=True)
            recb = smallp.tile([64, Q], f32, tag="recb")
            nc.vector.tensor_copy(recb[:], prb[:])
            ahead = smallp.tile([64, QPAD], bf16, tag="ahead")
            nc.vector.tensor_mul(ahead[:, :Q], po[0:64, :], recb[:])
            nc.vector.tensor_scalar_add(ahead[:, :Q], ahead[:, :Q], vb[:])
            nc.vector.memset(ahead[:, Q:], 0.0)
            cc_in_v = cc_in.ap().rearrange("(a p) f -> a p f", p=64)
            for j in range(8):
                nc.sync.dma_start(cc_in_v[j], ahead[:, j * QB:(j + 1) * QB])
            nc.gpsimd.collective_compute(
                "AllToAll", mybir.AluOpType.bypass,
                ins=[cc_in[:]], outs=[cc_out[:]], replica_groups=RG,
            )

            # ---- phase 7: final stage (replicated) -----------------------
            cc_view = cc_out.ap().rearrange("(a p) f -> a p f", p=128)
            ats = []
            for i in range(4):
                at = smallp.tile([128, QB], bf16, tag=f"at{i}")
                nc.sync.dma_start(at[:], cc_view[i])
                ats.append(at)

            pz = ps_o.tile([128, QB], f32, tag="o")
            for i in range(4):
                nc.tensor.matmul(pz[:], projT[:, i, :], ats[i][:],
                                 start=(i == 0), stop=(i == 3), skip_group_check=True)
            z1 = smallp.tile([128, QB], f32, tag="z1")
            nc.vector.tensor_add(z1[:], pz[:], x_blk[:])
            nc.vector.tensor_scalar_add(z1[:], z1[:], proj_b[:])

            def part_ln(z, g, b, out_dt, out_tag):
                """LayerNorm over the partition axis of z (128, QB)."""
                zsq = smallp.tile([128, QB], f32, tag="ln_sq")
                nc.vector.tensor_mul(zsq[:], z[:], z[:])
                p_sq = ps_s.tile([1, 2 * QB], f32, tag="s")
                p_s = p_sq[:, 0:QB]
                p_q = p_sq[:, QB:2 * QB]
                nc.tensor.matmul(p_s[:], ones128x1_f32[:], z[:],
                                 start=True, stop=True, skip_group_check=True)
                nc.tensor.matmul(p_q[:], ones128x1_f32[:], zsq[:],
                                 start=True, stop=True, skip_group_check=True)
                mu = smallp.tile([1, QB], f32, tag="ln_mu")
                nc.vector.tensor_scalar_mul(mu[:], p_s[:], 1.0 / DIM)
                musq = smallp.tile([1, QB], f32, tag="ln_musq")
                nc.vector.tensor_mul(musq[:], mu[:], mu[:])
                var = smallp.tile([1, QB], f32, tag="ln_var")
                nc.vector.tensor_scalar_mul(var[:], p_q[:], 1.0 / DIM)
                nc.vector.tensor_sub(var[:], var[:], musq[:])
                sig = smallp.tile([1, QB], f32, tag="ln_sig")
                nc.scalar.activation(sig[:], var[:],
                                     mybir.ActivationFunctionType.Sqrt, bias=eps1[:])
                rinv = smallp.tile([1, QB], f32, tag="ln_rinv")
                nc.vector.reciprocal(rinv[:], sig[:])
                mur = smallp.tile([1, QB], f32, tag="ln_mur")
                nc.vector.tensor_mul(mur[:], mu[:], rinv[:])
                pbb = ps_s.tile([128, 2 * QB], f32, tag="s")
                pb1 = pbb[:, 0:QB]
                pb2 = pbb[:, QB:2 * QB]
                nc.tensor.matmul(pb1[:], ones1x128[:], rinv[:], start=True, stop=True)
                nc.tensor.matmul(pb2[:], ones1x128[:], mur[:], start=True, stop=True)
                t1 = smallp.tile([128, QB], f32, tag="ln_t1")
                nc.vector.tensor_mul(t1[:], pb1[:], z[:])
                t2 = smallp.tile([128, QB], f32, tag="ln_t2")
                nc.vector.tensor_copy(t2[:], pb2[:])
                zl = smallp.tile([128, QB], out_dt, tag=out_tag)
                nc.vector.tensor_sub(t1[:], t1[:], t2[:])
                nc.vector.tensor_scalar(zl[:], t1[:], g[:], b[:],
                                        mybir.AluOpType.mult, mybir.AluOpType.add)
                return zl

            z_ln = part_ln(z1, pre_g, pre_b, f32, "zln")
            z_ln_bf = smallp.tile([128, QB], bf16, tag="zlnbf")
            nc.vector.tensor_copy(z_ln_bf[:], z_ln[:])

            h1s = []
            for j in range(2):
                ph = ps_s.tile([128, QB], f32, tag="s")
                nc.tensor.matmul(ph[:], mlp1T[:, j, :], z_ln_bf[:],
                                 start=True, stop=True)
                h1 = smallp.tile([128, QB], bf16, tag=f"h1{j}")
                nc.scalar.activation(h1[:], ph[:],
                                     mybir.ActivationFunctionType.Gelu,
                                     bias=mlp1_b[:, j:j + 1])
                h1s.append(h1)
            pz2 = ps_o.tile([128, QB], f32, tag="o")
            for j in range(2):
                nc.tensor.matmul(pz2[:], mlp2T[:, j, :], h1s[j][:],
                                 start=(j == 0), stop=(j == 1), skip_group_check=True)
            z2 = smallp.tile([128, QB], f32, tag="z2")
            nc.vector.tensor_scalar_add(z2[:], pz2[:], mlp2_b[:])
            nc.vector.tensor_add(z2[:], z2[:], z_ln[:])

            z_out = part_ln(z2, post_g, post_b, f32, "zout")
            nc.sync.dma_start(out_d.ap(), z_out[:])

    nc.compile()
    return nc


def _make_in_maps(dev):
    shared = {k: dev[k] for k in [
        'feature', 'img_c', 'proj_negc', 'lin_negc', 'proj_WcT', 'lin_WcT',
        'lin_bias', 'qsum_ln', 'projT', 'proj_b', 'mlp1T', 'mlp1_b',
        'mlp2T', 'mlp2_b', 'pre_g', 'pre_b', 'post_g', 'post_b']}
    in_maps = []
    for h in range(HEADS):
        m = dict(shared)
        m.update(dev['heads'][h])
        m['x_blk'] = np.ascontiguousarray(dev['x_pad'][:, h * QB:(h + 1) * QB])
        in_maps.append(m)
    return in_maps


def kernel(x, feature, I_inv, E_inv, params):
    import sys
    if '/opt/trn_rl_repo' not in sys.path:
        sys.path.insert(0, '/opt/trn_rl_repo')
    from concourse.bass_utils import run_bass_kernel_spmd

    dev = _host_prep(x, feature, I_inv, E_inv, params)
    if 'nc' not in _CACHE:
        _CACHE['nc'] = _build_program()
    nc = _CACHE['nc']

    in_maps = _make_in_maps(dev)

    res = run_bass_kernel_spmd(nc, in_maps, core_ids=list(range(8)))
    z = np.concatenate([np.asarray(res.results[c]['out'], np.float32)
                        for c in range(HEADS)], axis=1)[:, :Q]
    return z.reshape(1, DIM, H, W)
